# revision 17
# baseline (speedup 1.0000x reference)
"""Multi-head attention (B=8, N=2048, C=512, H=8, D=64) on 8 trn2 NeuronCores.

Sharding: data-parallel over batch — core b handles batch element b.
Dataflow (per core), all matmuls in float32r (full PE rate, ~1.5e-4 rel err):
  - host pre-transposes x -> xT [C, N] and weights -> wT [C, E]
  - QKV: qT/kT [d, n] chunks and V [m, 512] via f32r matmuls
  - scores: S^T[m, n] = K^T.T @ Q^T per head, two heads packed via PE row tiling
  - softmax: exp on ACT directly PSUM->SBUF (scale + per-key mask bias fused),
    denominator via a ones-column appended to V (M=65 PV matmul)
  - PV: out^T[d, n] accumulated in PSUM over key chunks
  - normalize: pack denominators via DMA, DVE reciprocal, DRAM-bounce
    partition-broadcast, one multiply per head-pair (overlapped per den half)
  - proj: y[n, o] f32r matmul + fused bias add
"""
import numpy as np

import concourse.bass as bass
import concourse.tile as tile
from concourse import bacc, mybir
from concourse.bass_utils import run_bass_kernel_spmd

F32 = mybir.dt.float32
F32R = mybir.dt.float32r
AF = mybir.ActivationFunctionType

B, N, C, H, D = 8, 2048, 512, 8, 64
SCALE = float(D) ** -0.5
NT = 512            # attention n-tile (psum moving width)
NNT = N // NT       # 4
MC = N // 128       # 16 key chunks
CC = C // 128       # 4 channel chunks
NP = H // 2         # 4 head pairs


def _bcast_ap(ap, nparts):
    """Partition-broadcast view of a single-partition (DRAM) AP."""
    return bass.AP(tensor=ap.tensor, offset=ap.offset, ap=[[0, nparts]] + list(ap.ap[1:]))


def _rows_ap(tile_, step, count):
    """AP selecting `count` partitions at stride `step` from a tile."""
    a = tile_[:]
    return bass.AP(tensor=a.tensor, offset=a.offset, ap=[[step, count]] + list(a.ap[1:]))


def build_body(nc, tc, ctx, xT, wqkvT, wpT, pbias, mb, y, rep=0, dbg=None):
    persist = ctx.enter_context(tc.tile_pool(name="persist", bufs=1))

    mb_sb = persist.tile([128, MC], F32)
    nc.sync.dma_start(mb_sb, mb[:])

    ones8 = persist.tile([128, H], F32)
    nc.vector.memset(ones8, 1.0)
    ones8_r = persist.tile([128, H], F32R)
    nc.vector.tensor_copy(ones8_r, ones8)

    qT = [persist.tile([128, N], F32R, name=f"qT{i}") for i in range(CC)]
    kT = [persist.tile([128, N], F32R, name=f"kT{i}") for i in range(CC)]
    v_sb = [persist.tile([128, H * 65], F32R, name=f"v{i}") for i in range(MC)]
    wp_r = [persist.tile([128, C], F32R, name=f"wp{i}") for i in range(CC)]
    dram_pool = ctx.enter_context(tc.tile_pool(name="dram", bufs=1, space="DRAM"))
    dram_den = dram_pool.tile([8, N], F32)

    # ---------------- phase 1: load + cast + QKV ----------------
    with (
        tc.tile_pool(name="qkv_sb", bufs=1) as qkv_sb,
        tc.tile_pool(name="stage", bufs=2) as stage,
        tc.tile_pool(name="qkv_ps", bufs=4, space="PSUM") as qkv_ps,
    ):
        wq_r = [qkv_sb.tile([128, 3 * C], F32R, name=f"wqkv{i}") for i in range(CC)]
        xT_r = [qkv_sb.tile([128, N], F32R, name=f"xTr{i}") for i in range(CC)]
        for cc in range(CC):
            t = stage.tile([128, N], F32, tag="ldstage")
            nc.sync.dma_start(t[:, :3 * C], wqkvT[cc * 128:(cc + 1) * 128, :])
            nc.vector.tensor_copy(wq_r[cc], t[:, :3 * C])
        for cc in range(CC):
            t = stage.tile([128, N], F32, tag="ldstage")
            nc.sync.dma_start(t[:, :C], wpT[cc * 128:(cc + 1) * 128, :])
            nc.vector.tensor_copy(wp_r[cc], t[:, :C])
        for cc in range(CC):
            t = stage.tile([128, N], F32, tag="ldstage")
            nc.sync.dma_start(t, xT[cc * 128:(cc + 1) * 128, :])
            nc.vector.tensor_copy(xT_r[cc], t)

        # ones columns of V' (65th column per head)
        for mc in range(MC):
            nc.vector.tensor_copy(
                v_sb[mc].rearrange("p (h e) -> p h e", h=H)[:, :, 64:65],
                ones8_r[:, :, None],
            )

        # Q^T and K^T: [d-chunk, n] = W^T.T @ x^T
        for which, dst in ((0, qT), (1, kT)):
            for mo in range(CC):
                for nt in range(NNT):
                    p = qkv_ps.tile([128, NT], F32, tag="qkps")
                    for kc in range(CC):
                        nc.tensor.matmul(
                            p,
                            lhsT=wq_r[kc][:, which * C + mo * 128: which * C + (mo + 1) * 128],
                            rhs=xT_r[kc][:, nt * NT:(nt + 1) * NT],
                            start=(kc == 0), stop=(kc == CC - 1),
                        )
                    nc.vector.tensor_copy(dst[mo][:, nt * NT:(nt + 1) * NT], p)

        # V: [m-chunk, 512] = x^T.T @ Wv^T, written strided into 65-wide head slots
        for mc in range(MC):
            p = qkv_ps.tile([128, C], F32, tag="vps")
            for kc in range(CC):
                nc.tensor.matmul(
                    p,
                    lhsT=xT_r[kc][:, mc * 128:(mc + 1) * 128],
                    rhs=wq_r[kc][:, 2 * C:3 * C],
                    start=(kc == 0), stop=(kc == CC - 1),
                )
            nc.vector.tensor_copy(
                v_sb[mc].rearrange("p (h e) -> p h e", h=H)[:, :, 0:64],
                p.rearrange("p (h d) -> p h d", h=H),
            )

    if dbg is not None:
        nc.sync.dma_start(dbg["qT0"][:], qT[0].bitcast(F32))
        nc.sync.dma_start(dbg["kT0"][:], kT[0].bitcast(F32))
        nc.sync.dma_start(dbg["v0"][:], v_sb[0].bitcast(F32))

    # ---------------- phases 2+3 ----------------
    with tc.tile_pool(name="long_sb", bufs=1) as long_sb:
        outT_r = [long_sb.tile([128, N], F32R, name=f"outTr{i}") for i in range(NP)]
        denP = long_sb.tile([128, 128], F32)
        denPr = long_sb.tile([128, 128], F32)
        pbias_bc = long_sb.tile([128, C], F32)
        nc.sync.dma_start(pbias_bc, pbias[:].to_broadcast([128, C]))

        # ---------------- phase 2: attention (+ overlapped normalize) -------
        with (
            tc.tile_pool(name="att_sb", bufs=3) as att_sb,
            tc.tile_pool(name="den_sb", bufs=1) as den_sb,
            tc.tile_pool(name="rbc_sb", bufs=2) as rbc_sb,
            tc.tile_pool(name="st_ps", bufs=2, space="PSUM") as st_ps,
            tc.tile_pool(name="pv_ps", bufs=2, space="PSUM") as pv_ps,
        ):
            # denominator rows parked at 32-aligned partitions (engine
            # partition-base alignment): den1 = heads 0-3 at {0,32,64,96}.
            den1 = den_sb.tile([128, N], F32)
            den2 = den1
            outT_u = [den_sb.tile([128, N], F32, name=f"outTu{i}") for i in range(NP)]

            def attention_pair(p_i):
                hA, hB = 2 * p_i, 2 * p_i + 1
                for nt in range(NNT):
                    nsl = slice(nt * NT, (nt + 1) * NT)
                    pvA = pv_ps.tile([65, NT], F32, tag="pvA")
                    pvB = pv_ps.tile([65, NT], F32, tag="pvB")
                    for mc in range(MC):
                        st = st_ps.tile([128, 2 * NT], F32, tag="st")
                        nc.tensor.matmul(
                            st[:, 0:NT],
                            lhsT=kT[p_i][0:64, mc * 128:(mc + 1) * 128],
                            rhs=qT[p_i][0:64, nsl],
                            start=True, stop=True, tile_position=(0, 0),
                        )
                        nc.tensor.matmul(
                            st[:, NT:2 * NT],
                            lhsT=kT[p_i][64:128, mc * 128:(mc + 1) * 128],
                            rhs=qT[p_i][64:128, nsl],
                            start=True, stop=True, tile_position=(64, 0),
                        )
                        pt = att_sb.tile([128, 2 * NT], F32R, tag="pt")
                        nc.scalar.activation(
                            pt, st, AF.Exp, scale=SCALE, bias=mb_sb[:, mc:mc + 1]
                        )
                        nc.tensor.matmul(
                            pvA, lhsT=v_sb[mc][:, hA * 65:(hA + 1) * 65],
                            rhs=pt[:, 0:NT],
                            start=(mc == 0), stop=(mc == MC - 1),
                        )
                        nc.tensor.matmul(
                            pvB, lhsT=v_sb[mc][:, hB * 65:(hB + 1) * 65],
                            rhs=pt[:, NT:2 * NT],
                            start=(mc == 0), stop=(mc == MC - 1),
                        )
                    # denominator rows (1-lane copies, 32-aligned dests)
                    dtile = den1 if p_i < 2 else den2
                    nc.vector.tensor_copy(
                        dtile[(hA % 4) * 32:(hA % 4) * 32 + 1, nsl], pvA[64:65, :]
                    )
                    nc.vector.tensor_copy(
                        dtile[(hB % 4) * 32:(hB % 4) * 32 + 1, nsl], pvB[64:65, :]
                    )
                    # unnormalized out^T rows
                    nc.vector.tensor_copy(outT_u[p_i][0:64, nsl], pvA[0:64, :])
                    nc.vector.tensor_copy(outT_u[p_i][64:128, nsl], pvB[0:64, :])

            def recip_half(half):
                """pack den half -> reciprocal -> DRAM bounce."""
                dtile = den1
                po = half * 64
                nc.sync.dma_start(denP[po:po + 64, :], dtile[0:128:32, :])
                nc.vector.reciprocal(denPr[po:po + 64, :], denP[po:po + 64, :])
                nc.sync.dma_start(dram_den[half * 4:half * 4 + 4, :],
                                  denPr[po:po + 64, :])

            def normalize_pair(p_i):
                hA, hB = 2 * p_i, 2 * p_i + 1
                for nt in range(NNT):
                    nsl = slice(nt * NT, (nt + 1) * NT)
                    rbc = rbc_sb.tile([128, NT], F32, tag="rbc")
                    nc.sync.dma_start(
                        rbc[0:64, :], dram_den[hA:hA + 1, nsl].to_broadcast([64, NT]))
                    nc.sync.dma_start(
                        rbc[64:128, :], dram_den[hB:hB + 1, nsl].to_broadcast([64, NT]))
                    nc.vector.tensor_tensor(
                        outT_r[p_i][:, nsl], outT_u[p_i][:, nsl], rbc,
                        mybir.AluOpType.mult,
                    )

            attention_pair(0)
            if dbg is not None:
                nc.sync.dma_start(dbg["outTu0"][:], outT_u[0][:])
                nc.sync.dma_start(dbg["den1"][:], den1[:])
            attention_pair(1)
            recip_half(0)
            attention_pair(2)
            normalize_pair(0)
            normalize_pair(1)
            attention_pair(3)
            recip_half(1)
            normalize_pair(2)
            normalize_pair(3)

        if dbg is not None:
            nc.sync.dma_start(dbg["outTr0"][:], outT_r[0].bitcast(F32))
            nc.sync.dma_start(dbg["outTr3"][:], outT_r[3].bitcast(F32))
            nc.sync.dma_start(dbg["denPr"][:], denPr[:])
            nc.sync.dma_start(dbg["den2"][:], den2[:])
            nc.sync.dma_start(dbg["denP"][:], denP[:])
            nc.sync.dma_start(dbg["pbias_bc"][:], pbias_bc[:])

        # ---------------- phase 3: proj ----------------
        with (
            tc.tile_pool(name="proj_sb", bufs=3) as proj_sb,
            tc.tile_pool(name="proj_ps", bufs=4, space="PSUM") as proj_ps,
        ):
            for nc2 in range(MC):
                p = proj_ps.tile([128, C], F32, tag="yps")
                for cc in range(CC):
                    nc.tensor.matmul(
                        p,
                        lhsT=outT_r[cc][:, nc2 * 128:(nc2 + 1) * 128],
                        rhs=wp_r[cc],
                        start=(cc == 0), stop=(cc == CC - 1),
                    )
                ysb = proj_sb.tile([128, C], F32, tag="ysb")
                nc.vector.tensor_tensor(ysb, p, pbias_bc, mybir.AluOpType.add)
                nc.sync.dma_start(y[nc2 * 128:(nc2 + 1) * 128, :], ysb)


def build_nc(reps=1, debug_outs=False):
    nc = bacc.Bacc("TRN2", target_bir_lowering=False, debug=False)
    xT = nc.declare_dram_parameter("xT", [C, N], F32, isOutput=False)
    wqkvT = nc.declare_dram_parameter("wqkvT", [C, 3 * C], F32, isOutput=False)
    wpT = nc.declare_dram_parameter("wpT", [C, C], F32, isOutput=False)
    pbias = nc.declare_dram_parameter("pbias", [1, C], F32, isOutput=False)
    mb = nc.declare_dram_parameter("mb", [128, MC], F32, isOutput=False)
    y = nc.declare_dram_parameter("y", [N, C], F32, isOutput=True)
    dbg = None
    if debug_outs:
        dbg = {
            "qT0": nc.declare_dram_parameter("qT0", [128, N], F32, isOutput=True),
            "kT0": nc.declare_dram_parameter("kT0", [128, N], F32, isOutput=True),
            "v0": nc.declare_dram_parameter("v0", [128, H * 65], F32, isOutput=True),
            "outTu0": nc.declare_dram_parameter("outTu0", [128, N], F32, isOutput=True),
            "den1": nc.declare_dram_parameter("den1", [128, N], F32, isOutput=True),
            "outTr0": nc.declare_dram_parameter("outTr0", [128, N], F32, isOutput=True),
            "outTr3": nc.declare_dram_parameter("outTr3", [128, N], F32, isOutput=True),
            "denPr": nc.declare_dram_parameter("denPr", [128, 128], F32, isOutput=True),
            "den2": nc.declare_dram_parameter("den2", [128, N], F32, isOutput=True),
            "denP": nc.declare_dram_parameter("denP", [128, 128], F32, isOutput=True),
            "pbias_bc": nc.declare_dram_parameter("pbias_bc", [128, C], F32, isOutput=True),
        }
    from contextlib import ExitStack
    with tile.TileContext(nc) as tc:
        for r in range(reps):
            with ExitStack() as ctx:
                build_body(nc, tc, ctx, xT, wqkvT, wpT, pbias, mb, y, rep=r, dbg=dbg)
    nc.finalize()
    return nc


def prep_inputs(x, mask, qkv_w, proj_w, proj_b):
    wqkvT = np.ascontiguousarray(qkv_w.T.astype(np.float32))
    wpT = np.ascontiguousarray(proj_w.T.astype(np.float32))
    pb = np.ascontiguousarray(proj_b.astype(np.float32).reshape(1, C))
    in_maps = []
    for b in range(B):
        bias = np.where(np.asarray(mask[b]), 0.0, -1e9).astype(np.float32)
        in_maps.append({
            "xT": np.ascontiguousarray(np.asarray(x[b]).T.astype(np.float32)),
            "wqkvT": wqkvT,
            "wpT": wpT,
            "pbias": pb,
            "mb": np.ascontiguousarray(bias.reshape(MC, 128).T),
        })
    return in_maps


_CACHED_NC = None


def kernel(x, mask, qkv_w, proj_w, proj_b):
    global _CACHED_NC
    if _CACHED_NC is None:
        _CACHED_NC = build_nc()
    in_maps = prep_inputs(x, mask, qkv_w, proj_w, proj_b)
    res = run_bass_kernel_spmd(_CACHED_NC, in_maps, list(range(B)))
    out = np.stack([res.results[b]["y"] for b in range(B)], axis=0)
    return out.astype(np.float32)
